# revision 9
# baseline (speedup 1.0000x reference)
"""v3: class-split scatter chains + multi-queue SWDGE for LSS voxel pooling.

Design (per core = (batch b, grid half h)):
  - segments are partitioned into NCLS=4 classes by s % 4; per (camera,
    class) the nonempty segments are count-sorted and packed into
    128-segment windows (tight L), giving per-(cam,class) chunks.
  - chunks are emitted round-robin over classes; each chunk:
      gather (SWDGE queue = class) -> tensor_reduce all windows into a
      contiguous result buffer -> dma_scatter_add into that class's own
      HBM grid [5001, 64] (queue = class).
    Four class grids = four independent WAW scatter chains that run
    concurrently on four SWDGE queues (disjoint segment sets, so no
    cross-chain races; within a chain Tile's WAW sems serialize).
  - scatter emission lags gather emission by LAG chunks so scatter's
    data-dependency waits are pre-satisfied and never stall the in-order
    Pool engine descriptor-generation stream.
  - host de-interleaves the 4 class grids into the final output.
"""

import sys

import numpy as np

sys.path.insert(0, "/opt/trn_rl_repo")


def _patch_dmasw_lane_per_queue():
    """Pin each SWDGE queue to its own pair of DMASW sem lanes.

    Tile's default round-robin over 8 lanes lets one sem serve DMAs from
    different SWDGE queues; CoreSim locks each sem to one queue and errors.
    Mapping lane = queue*2 + toggle keeps sems queue-pure (HW-neutral).
    """
    from concourse import tile_sem_assignment as tsa

    if getattr(tsa, "_lane_per_queue_patch", False):
        return
    tsa._lane_per_queue_patch = True
    orig = tsa.TileClockTick._assign_tick
    import concourse.mybir as mybir

    def _assign_tick(self, inst):
        q = getattr(inst, "queue_num", None)
        if (
            q is not None
            and inst.engine == mybir.EngineType.Pool
            and isinstance(inst, tsa.DMAInst)
        ):
            self.next_sw_dma_idx = q * 2
        return orig(self, inst)

    tsa.TileClockTick._assign_tick = _assign_tick


_patch_dmasw_lane_per_queue()

# ---- problem constants ----
B, N, D, H, W, C = 4, 6, 41, 16, 44, 64
NX, NY, NZ = 200, 200, 1
CAM_ROWS = D * H * W          # 28864
NSEG = NX * NY * NZ           # 40000
NSEG_H = NSEG // 2            # 20000 per core
ZROW = CAM_ROWS               # per-camera zero row
PART = 128
CH_TILES = 64                 # max 128-row tiles per gather chunk
N_CORES = 8
NCLS = 4                      # segment classes (s % NCLS) = scatter chains
NSEG_CL = NSEG_H // NCLS      # 5000 per class grid
DUMP_CL = NSEG_CL             # dump row in class grid
LAG = 5                       # scatter emission lag (chunks)

LAST_RESULTS = None


def _wrap16(lst):
    n = len(lst)
    assert n % 16 == 0
    w = lst.reshape(n // 16, 16).T.astype(np.int16)
    return np.tile(w, (8, 1))


def _host_core(geom_b, h):
    """Per-core counting sorts + per-(cam,class) count-sorted windows."""
    g = geom_b.reshape(N, CAM_ROWS, 3).astype(np.int64)
    cams = []
    for c in range(N):
        gx, gy, gz = g[c, :, 0], g[c, :, 1], g[c, :, 2]
        kept = (gx >= 0) & (gx < NX) & (gy >= 0) & (gy < NY) & (gz >= 0) & (gz < NZ)
        seg = gx * (NY * NZ) + gy * NZ + gz
        sel = kept & (seg >= h * NSEG_H) & (seg < (h + 1) * NSEG_H)
        rows = np.nonzero(sel)[0].astype(np.int32)
        sl = (seg[sel] - h * NSEG_H).astype(np.int32)
        order = np.argsort(sl, kind="stable")
        rows_sorted = rows[order]
        cnt = np.bincount(sl, minlength=NSEG_H).astype(np.int32)
        if cnt.max(initial=0) > PART:
            return None
        starts = np.zeros(NSEG_H, np.int64)
        np.cumsum(cnt[:-1], out=starts[1:])
        wins = []
        for k in range(NCLS):
            segs = np.nonzero(cnt)[0]
            segs = segs[segs % NCLS == k]
            order2 = np.lexsort((segs, -cnt[segs]))
            segs = segs[order2]
            n = len(segs)
            Wn = (n + PART - 1) // PART
            tmp = np.full(Wn * PART, -1, np.int64)
            tmp[:n] = segs
            grid = np.ascontiguousarray(tmp.reshape(Wn, PART).T)  # [128, Wn]
            lw = (cnt[tmp.reshape(Wn, PART)[:, 0]] if Wn
                  else np.zeros(0, np.int32))
            wins.append((grid, lw.astype(np.int32)))
        cams.append(dict(rows_sorted=rows_sorted, cnt=cnt, starts=starts,
                         wins=wins))
    return dict(cams=cams)


def _plan(cores):
    """Unified cross-core chunk plan.

    Returns chunks, each dict(cam, cls, runs, tiles, nw, gt, gw) where
    runs = [(L, wstart, nw_run, tile_off, wcol_off)] over the unified
    per-(cam,class) window L sequence.  Chunk order: per cam,
    round-robin over classes (chain overlap)."""
    lw_cc = {}
    for c in range(N):
        for k in range(NCLS):
            Wn = max(cr["cams"][c]["wins"][k][1].shape[0] for cr in cores)
            lw = np.ones(Wn, np.int32)
            for cr in cores:
                l = cr["cams"][c]["wins"][k][1]
                lw[: len(l)] = np.maximum(lw[: len(l)], l)
            lw_cc[(c, k)] = lw

    # build per-(cam,class) chunk lists
    cc_chunks = {}
    for (c, k), lw in lw_cc.items():
        lst = []
        cur_runs, cur_tiles, cur_w = [], 0, 0

        def flush():
            nonlocal cur_runs, cur_tiles, cur_w
            if cur_runs:
                lst.append(dict(cam=c, cls=k, runs=cur_runs,
                                tiles=cur_tiles, nw=cur_w))
            cur_runs, cur_tiles, cur_w = [], 0, 0

        w = 0
        Wn = len(lw)
        while w < Wn:
            L = int(lw[w])
            nw_same = 1
            while w + nw_same < Wn and lw[w + nw_same] == L:
                nw_same += 1
            taken = 0
            while taken < nw_same:
                room = (CH_TILES - cur_tiles) // L
                if room == 0:
                    flush()
                    room = CH_TILES // L
                take = min(room, nw_same - taken)
                cur_runs.append((L, w + taken, take, cur_tiles, cur_w))
                cur_tiles += take * L
                cur_w += take
                taken += take
            w += nw_same
        flush()
        cc_chunks[(c, k)] = lst

    # interleave: per cam, round-robin over classes
    chunks = []
    for c in range(N):
        lists = [list(cc_chunks[(c, k)]) for k in range(NCLS)]
        i = 0
        while any(lists):
            k = i % NCLS
            if lists[k]:
                chunks.append(lists[k].pop(0))
            i += 1

    gt = gw = 0
    for ch in chunks:
        ch["gt"], ch["gw"] = gt, gw
        gt += ch["tiles"]
        gw += ch["nw"]
    return dict(lw_cc=lw_cc, chunks=chunks, tot_tiles=gt, tot_w=gw,
                max_tiles=max((ch["tiles"] for ch in chunks), default=1),
                max_nw=max((ch["nw"] for ch in chunks), default=1))


def _core_indices(core, plan):
    """Per-core int16 gather/scatter token lists in plan order."""
    tot_tiles, tot_w = plan["tot_tiles"], plan["tot_w"]
    glist = np.full(tot_tiles * PART, ZROW, np.int16)
    slist = np.full(tot_w * PART, DUMP_CL, np.int16)
    for ch in plan["chunks"]:
        cam = core["cams"][ch["cam"]]
        rows_sorted, cnt, starts = cam["rows_sorted"], cam["cnt"], cam["starts"]
        grid_full = cam["wins"][ch["cls"]][0]
        nsrc = max(len(rows_sorted), 1)
        for L, w0, nw, toff, woff in ch["runs"]:
            grid = np.full((PART, nw), -1, np.int64)
            avail = max(0, min(nw, grid_full.shape[1] - w0))
            if avail:
                grid[:, :avail] = grid_full[:, w0 : w0 + avail]
            seg = grid.T                      # [nw, 128]
            real = seg >= 0
            segc = np.where(real, seg, 0)
            st = starts[segc]
            kc = np.where(real, cnt[segc], 0)
            j = np.arange(L)[:, None, None]
            m = st[None] + j
            vj = j < kc[None]
            rows = np.where(vj, rows_sorted[np.minimum(m, nsrc - 1)], ZROW)
            base = (ch["gt"] + toff) * PART
            glist[base : base + nw * L * PART] = (
                rows.transpose(1, 0, 2).reshape(-1).astype(np.int16)
            )
            sbase = (ch["gw"] + woff) * PART
            slist[sbase : sbase + nw * PART] = (
                np.where(real, seg // NCLS, DUMP_CL).reshape(-1).astype(np.int16)
            )
    return _wrap16(glist), _wrap16(slist)


def _host_layout(x, geom_feats):
    geom = np.asarray(geom_feats)
    cores = []
    for b in range(B):
        for h in range(2):
            cr = _host_core(geom[b], h)
            if cr is None:
                return None, None, None
            cr["b"], cr["h"] = b, h
            cores.append(cr)
    plan = _plan(cores)
    per_core = []
    for cr in cores:
        gi, si = _core_indices(cr, plan)
        per_core.append(dict(b=cr["b"], h=cr["h"], gidx=gi, sidx=si))
    return plan, cores, per_core


def _build_program(plan, loop_reps=None, parts=("gather", "reduce", "scatter")):
    from concourse import bacc, mybir, tile

    nc = bacc.Bacc("TRN2", target_bir_lowering=False, debug=False,
                   num_devices=N_CORES, dynamic_dma_scratch_size=65536,
                   num_swdge_queues=NCLS)
    f32 = mybir.dt.float32
    i16 = mybir.dt.int16

    tot_tiles, tot_w = plan["tot_tiles"], plan["tot_w"]
    xz = [
        nc.dram_tensor(f"xz{c}", [CAM_ROWS + 1, C], f32, kind="ExternalInput")
        for c in range(N)
    ]
    gidx_d = nc.dram_tensor("gidx", [PART, tot_tiles * 8], i16,
                            kind="ExternalInput")
    sidx_d = nc.dram_tensor("sidx", [PART, tot_w * 8], i16,
                            kind="ExternalInput")
    outs = [
        nc.dram_tensor(f"out{k}", [NSEG_CL + 1, C], f32,
                       kind="ExternalOutput")
        for k in range(NCLS)
    ]
    store_d = (
        nc.dram_tensor("partials", [tot_w * PART, C], f32,
                       kind="ExternalOutput")
        if "store" in parts
        else None
    )

    gb_cols = plan["max_tiles"] * C
    rb_cols = plan["max_nw"] * C

    with tile.TileContext(nc) as tc:
        with (
            tc.tile_pool(name="idxp", bufs=1) as ip,
            tc.tile_pool(name="gatp", bufs=LAG + 2) as gp,
            tc.tile_pool(name="resp", bufs=LAG + 2) as rp,
        ):
            gidx_sb = ip.tile([PART, tot_tiles * 8], i16)
            sidx_sb = ip.tile([PART, tot_w * 8], i16)
            # split the gidx load so the first gathers only wait on
            # their own slice (cuts single-shot pipeline-fill latency)
            head = min(plan["chunks"][1]["gt"] + plan["chunks"][1]["tiles"]
                       if len(plan["chunks"]) > 1 else tot_tiles,
                       tot_tiles) * 8
            nc.sync.dma_start(out=gidx_sb[:, :head], in_=gidx_d[:, :head])
            if head < tot_tiles * 8:
                nc.sync.dma_start(out=gidx_sb[:, head:],
                                  in_=gidx_d[:, head:])
            nc.sync.dma_start(out=sidx_sb[:], in_=sidx_d[:])

            def body():
                _emit(nc, tc, plan, xz, gidx_sb, sidx_sb, outs, gp, rp,
                      gb_cols, rb_cols, mybir, parts=parts, store_d=store_d)

            if loop_reps:
                with tc.For_i(0, loop_reps, 1):
                    body()
            else:
                body()

    nc.compile()
    return nc


def _emit(nc, tc, plan, xz, gidx_sb, sidx_sb, outs, gp, rp,
          gb_cols, rb_cols, mybir, parts=("gather", "reduce", "scatter"),
          store_d=None):
    from concourse import mybir as _mb
    f32 = _mb.dt.float32
    pend = []

    def emit_scatter(ch, res):
        nw = ch["nw"]
        n_tok = nw * PART
        nc.gpsimd.dma_scatter_add(
            out_ap=outs[ch["cls"]][:, :],
            in_ap=res[:, : nw * C].rearrange("p (t c) -> p t c", t=nw, c=C),
            idxs_ap=sidx_sb[:, ch["gw"] * 8 : (ch["gw"] + nw) * 8],
            num_idxs=n_tok,
            num_idxs_reg=n_tok,
            elem_size=C,
            single_packet=True,
            queue_num=ch["cls"],
        )

    for ch in plan["chunks"]:
        T, nw = ch["tiles"], ch["nw"]
        buf = gp.tile([PART, gb_cols], f32, tag="gbuf")
        n_in = T * PART
        nc.gpsimd.dma_gather(
            out_ap=buf[:, : T * C].rearrange("p (t c) -> p t c", t=T, c=C),
            in_ap=xz[ch["cam"]][:, :],
            idxs_ap=gidx_sb[:, ch["gt"] * 8 : (ch["gt"] + T) * 8],
            num_idxs=n_in,
            num_idxs_reg=n_in,
            elem_size=C,
            single_packet=False,
            queue_num=ch["cls"],
        )
        if "reduce" not in parts:
            continue
        res = rp.tile([PART, rb_cols], f32, tag="rbuf")
        for L, _ws, nwr, toff, woff in ch["runs"]:
            view = buf[:, toff * C : (toff + nwr * L) * C].rearrange(
                "p (w j c) -> p w c j", w=nwr, j=L, c=C)
            nc.vector.tensor_reduce(
                out=res[:, woff * C : (woff + nwr) * C],
                in_=view,
                axis=mybir.AxisListType.X,
                op=mybir.AluOpType.add,
            )
        if "store" in parts:
            nw = ch["nw"]
            tot_w = plan["tot_w"]
            store_v = store_d[:, :].rearrange(
                "(p t) c -> p t c", p=PART, t=tot_w)
            nc.sync.dma_start(
                out=store_v[:, ch["gw"] : ch["gw"] + nw, :],
                in_=res[:, : nw * C].rearrange("p (t c) -> p t c", t=nw, c=C),
            )
        if "scatter" not in parts:
            continue
        pend.append((ch, res))
        if len(pend) > LAG:
            emit_scatter(*pend.pop(0))
    for ch, res in pend:
        emit_scatter(ch, res)


def _numpy_fallback(x, geom_feats):
    feats = np.asarray(x).reshape(-1, C)
    g = np.asarray(geom_feats).reshape(-1, 3).astype(np.int64)
    npr = feats.shape[0]
    batch_ix = np.repeat(np.arange(B, dtype=np.int64), npr // B)
    kept = (
        (g[:, 0] >= 0) & (g[:, 0] < NX)
        & (g[:, 1] >= 0) & (g[:, 1] < NY)
        & (g[:, 2] >= 0) & (g[:, 2] < NZ)
    )
    feats = np.where(kept[:, None], feats, 0.0)
    seg = batch_ix * NSEG + g[:, 0] * NY * NZ + g[:, 1] * NZ + g[:, 2]
    seg = np.where(kept, seg, batch_ix * NSEG)
    pooled = np.zeros((B * NSEG, C), np.float32)
    np.add.at(pooled, seg, feats)
    grid = pooled.reshape(B, NX, NY, NZ, C).transpose(0, 4, 3, 1, 2)
    return np.ascontiguousarray(grid.reshape(B, C * NZ, NX, NY))


def _make_in_maps(x, per_core):
    zero_row = np.zeros((1, C), np.float32)
    xz_by_batch = []
    for b in range(B):
        xb = x[b].reshape(N, CAM_ROWS, C)
        xz_by_batch.append(
            [np.concatenate([xb[c], zero_row], axis=0) for c in range(N)]
        )

    in_maps = []
    for pc in per_core:
        m = {f"xz{c}": xz_by_batch[pc["b"]][c] for c in range(N)}
        m["gidx"] = pc["gidx"]
        m["sidx"] = pc["sidx"]
        in_maps.append(m)
    return in_maps


def kernel(x, geom_feats):
    from concourse import bass_utils

    x = np.ascontiguousarray(np.asarray(x, dtype=np.float32))
    plan, cores, per_core = _host_layout(x, geom_feats)
    if plan is None:
        return _numpy_fallback(x, geom_feats)

    nc = _build_program(plan)
    in_maps = _make_in_maps(x, per_core)

    res = bass_utils.run_bass_kernel_spmd(
        nc, in_maps, core_ids=list(range(N_CORES))
    )
    global LAST_RESULTS
    LAST_RESULTS = res

    out = np.zeros((B, C * NZ, NX, NY), np.float32)
    half = np.empty((NSEG_H, C), np.float32)
    for pc, r in zip(per_core, res.results):
        for k in range(NCLS):
            half[k::NCLS] = r[f"out{k}"][:NSEG_CL]
        grid = half.reshape(NX // 2, NY, C)
        out[pc["b"], :, pc["h"] * (NX // 2) : (pc["h"] + 1) * (NX // 2), :] = (
            grid.transpose(2, 0, 1)
        )
    return out



# revision 18
# speedup vs baseline: 8.6240x; 8.6240x over previous
"""v3: class-split scatter chains + multi-queue SWDGE for LSS voxel pooling.

Design (per core = (batch b, grid half h)):
  - segments are partitioned into NCLS=4 classes by s % 4; per (camera,
    class) the nonempty segments are count-sorted and packed into
    128-segment windows (tight L), giving per-(cam,class) chunks.
  - chunks are emitted round-robin over classes; each chunk:
      gather (SWDGE queue = class) -> tensor_reduce all windows into a
      contiguous result buffer -> dma_scatter_add into that class's own
      HBM grid [5001, 64] (queue = class).
    Four class grids = four independent WAW scatter chains that run
    concurrently on four SWDGE queues (disjoint segment sets, so no
    cross-chain races; within a chain Tile's WAW sems serialize).
  - scatter emission lags gather emission by LAG chunks so scatter's
    data-dependency waits are pre-satisfied and never stall the in-order
    Pool engine descriptor-generation stream.
  - host de-interleaves the 4 class grids into the final output.
"""

import sys

import numpy as np

sys.path.insert(0, "/opt/trn_rl_repo")


def _patch_dmasw_lane_per_queue():
    """Pin each SWDGE queue to its own pair of DMASW sem lanes.

    Tile's default round-robin over 8 lanes lets one sem serve DMAs from
    different SWDGE queues; CoreSim locks each sem to one queue and errors.
    Mapping lane = queue*2 + toggle keeps sems queue-pure (HW-neutral).
    """
    from concourse import tile_sem_assignment as tsa

    if getattr(tsa, "_lane_per_queue_patch", False):
        return
    tsa._lane_per_queue_patch = True
    orig = tsa.TileClockTick._assign_tick
    import concourse.mybir as mybir

    def _assign_tick(self, inst):
        q = getattr(inst, "queue_num", None)
        if (
            q is not None
            and inst.engine == mybir.EngineType.Pool
            and isinstance(inst, tsa.DMAInst)
        ):
            self.next_sw_dma_idx = q * 2
        return orig(self, inst)

    tsa.TileClockTick._assign_tick = _assign_tick


_patch_dmasw_lane_per_queue()

# ---- problem constants ----
B, N, D, H, W, C = 4, 6, 41, 16, 44, 64
NX, NY, NZ = 200, 200, 1
CAM_ROWS = D * H * W          # 28864
NSEG = NX * NY * NZ           # 40000
NSEG_H = NSEG // 2            # 20000 per core
ZROW = CAM_ROWS               # per-camera zero row
PART = 128
CH_TILES = 64                 # max 128-row tiles per gather chunk
N_CORES = 8
NCLS = 4                      # segment classes (s % NCLS) = scatter chains
NSEG_CL = NSEG_H // NCLS      # 5000 per class grid
DUMP_CL = NSEG_CL             # dump row in class grid
LAG = 5                       # scatter emission lag (chunks)

LAST_RESULTS = None
# Trailing -1 trim is DISABLED: the NX decode reserves descriptor-ring
# space from num_idxs_reg (full count) while the Q7 writes fewer descs
# after its trailing-negative trim; the mismatch corrupts the ring and
# wedges the device after enough iterations.
TRIM = False


def _wrap16(lst):
    n = len(lst)
    assert n % 16 == 0
    w = lst.reshape(n // 16, 16).T.astype(np.int16)
    return np.tile(w, (8, 1))


def _host_core(geom_b, h):
    """Per-core counting sorts + per-(cam,class) count-sorted windows."""
    g = geom_b.reshape(N, CAM_ROWS, 3).astype(np.int64)
    cams = []
    for c in range(N):
        gx, gy, gz = g[c, :, 0], g[c, :, 1], g[c, :, 2]
        kept = (gx >= 0) & (gx < NX) & (gy >= 0) & (gy < NY) & (gz >= 0) & (gz < NZ)
        seg = gx * (NY * NZ) + gy * NZ + gz
        sel = kept & (seg >= h * NSEG_H) & (seg < (h + 1) * NSEG_H)
        rows = np.nonzero(sel)[0].astype(np.int32)
        sl = (seg[sel] - h * NSEG_H).astype(np.int32)
        order = np.argsort(sl, kind="stable")
        rows_sorted = rows[order]
        cnt = np.bincount(sl, minlength=NSEG_H).astype(np.int32)
        if cnt.max(initial=0) > PART:
            return None
        starts = np.zeros(NSEG_H, np.int64)
        np.cumsum(cnt[:-1], out=starts[1:])
        wins = []
        for k in range(NCLS):
            segs = np.nonzero(cnt)[0]
            segs = segs[segs % NCLS == k]
            # count-desc for tight windows; row-ascending within a count
            # class so the gather's HBM reads stream in address order
            first_row = rows_sorted[starts[segs]]
            order2 = np.lexsort((first_row, -cnt[segs]))
            segs = segs[order2]
            n = len(segs)
            Wn = (n + PART - 1) // PART
            tmp = np.full(Wn * PART, -1, np.int64)
            tmp[:n] = segs
            grid = np.ascontiguousarray(tmp.reshape(Wn, PART).T)  # [128, Wn]
            lw = (cnt[tmp.reshape(Wn, PART)[:, 0]] if Wn
                  else np.zeros(0, np.int32))
            wins.append((grid, lw.astype(np.int32)))
        cams.append(dict(rows_sorted=rows_sorted, cnt=cnt, starts=starts,
                         wins=wins))
    return dict(cams=cams)


def _plan(cores):
    """Unified cross-core chunk plan.

    Returns chunks, each dict(cam, cls, runs, tiles, nw, gt, gw) where
    runs = [(L, wstart, nw_run, tile_off, wcol_off)] over the unified
    per-(cam,class) window L sequence.  Chunk order: per cam,
    round-robin over classes (chain overlap)."""
    lw_cc = {}
    for c in range(N):
        for k in range(NCLS):
            Wn = max(cr["cams"][c]["wins"][k][1].shape[0] for cr in cores)
            lw = np.ones(Wn, np.int32)
            for cr in cores:
                l = cr["cams"][c]["wins"][k][1]
                lw[: len(l)] = np.maximum(lw[: len(l)], l)
            lw_cc[(c, k)] = lw

    # build per-(cam,class) chunk lists
    cc_chunks = {}
    for (c, k), lw in lw_cc.items():
        lst = []
        cur_runs, cur_tiles, cur_w = [], 0, 0

        def flush():
            nonlocal cur_runs, cur_tiles, cur_w
            if cur_runs:
                lst.append(dict(cam=c, cls=k, runs=cur_runs,
                                tiles=cur_tiles, nw=cur_w))
            cur_runs, cur_tiles, cur_w = [], 0, 0

        w = 0
        Wn = len(lw)
        while w < Wn:
            L = int(lw[w])
            nw_same = 1
            while w + nw_same < Wn and lw[w + nw_same] == L:
                nw_same += 1
            taken = 0
            while taken < nw_same:
                room = (CH_TILES - cur_tiles) // L
                if room == 0:
                    flush()
                    room = CH_TILES // L
                take = min(room, nw_same - taken)
                cur_runs.append((L, w + taken, take, cur_tiles, cur_w))
                cur_tiles += take * L
                cur_w += take
                taken += take
            w += nw_same
        flush()
        cc_chunks[(c, k)] = lst

    # interleave: per cam, round-robin over classes
    chunks = []
    for c in range(N):
        lists = [list(cc_chunks[(c, k)]) for k in range(NCLS)]
        i = 0
        while any(lists):
            k = i % NCLS
            if lists[k]:
                chunks.append(lists[k].pop(0))
            i += 1

    gt = gw = 0
    for ch in chunks:
        ch["gt"], ch["gw"] = gt, gw
        gt += ch["tiles"]
        gw += ch["nw"]
    return dict(lw_cc=lw_cc, chunks=chunks, tot_tiles=gt, tot_w=gw,
                max_tiles=max((ch["tiles"] for ch in chunks), default=1),
                max_nw=max((ch["nw"] for ch in chunks), default=1))


def _core_indices(core, plan):
    """Per-core int16 gather/scatter token lists in plan order."""
    tot_tiles, tot_w = plan["tot_tiles"], plan["tot_w"]
    glist = np.full(tot_tiles * PART, ZROW, np.int16)
    slist = np.full(tot_w * PART, DUMP_CL, np.int16)
    for ch in plan["chunks"]:
        cam = core["cams"][ch["cam"]]
        rows_sorted, cnt, starts = cam["rows_sorted"], cam["cnt"], cam["starts"]
        grid_full = cam["wins"][ch["cls"]][0]
        nw_core = grid_full.shape[1]      # this core's real window count
        nsrc = max(len(rows_sorted), 1)
        for L, w0, nw, toff, woff in ch["runs"]:
            grid = np.full((PART, nw), -1, np.int64)
            avail = max(0, min(nw, grid_full.shape[1] - w0))
            if avail:
                grid[:, :avail] = grid_full[:, w0 : w0 + avail]
            seg = grid.T                      # [nw, 128]
            real = seg >= 0
            segc = np.where(real, seg, 0)
            st = starts[segc]
            kc = np.where(real, cnt[segc], 0)
            j = np.arange(L)[:, None, None]
            m = st[None] + j
            vj = j < kc[None]
            rows = np.where(vj, rows_sorted[np.minimum(m, nsrc - 1)], ZROW)
            # windows this core doesn't have at all sit at the tail of the
            # chunk; mark their tokens -1 so the Q7 ucode's trailing-negative
            # trim skips descriptor generation for them on this core.  The
            # un-gathered SBUF is stale garbage, but those windows scatter to
            # the dump row only.
            absent = (np.arange(w0, w0 + nw) >= nw_core) if TRIM \
                else np.zeros(nw, bool)
            rows[:, absent, :] = -1
            base = (ch["gt"] + toff) * PART
            glist[base : base + nw * L * PART] = (
                rows.transpose(1, 0, 2).reshape(-1).astype(np.int16)
            )
            sbase = (ch["gw"] + woff) * PART
            stok = np.where(real, seg // NCLS, DUMP_CL)     # [nw, 128]
            stok[absent, :] = -1
            slist[sbase : sbase + nw * PART] = stok.reshape(-1).astype(np.int16)
    return _wrap16(glist), _wrap16(slist)


def _host_layout(x, geom_feats):
    geom = np.asarray(geom_feats)
    cores = []
    for b in range(B):
        for h in range(2):
            cr = _host_core(geom[b], h)
            if cr is None:
                return None, None, None
            cr["b"], cr["h"] = b, h
            cores.append(cr)
    plan = _plan(cores)
    per_core = []
    for cr in cores:
        gi, si = _core_indices(cr, plan)
        per_core.append(dict(b=cr["b"], h=cr["h"], gidx=gi, sidx=si))
    return plan, cores, per_core


def _build_program(plan, loop_reps=None, parts=("gather", "reduce", "scatter"),
                   queue_mode="cls", max_chunks=None):
    from concourse import bacc, mybir, tile

    nc = bacc.Bacc("TRN2", target_bir_lowering=False, debug=False,
                   num_devices=N_CORES, dynamic_dma_scratch_size=65536,
                   num_swdge_queues=NCLS)
    f32 = mybir.dt.float32
    i16 = mybir.dt.int16

    tot_tiles, tot_w = plan["tot_tiles"], plan["tot_w"]
    xz = [
        nc.dram_tensor(f"xz{c}", [CAM_ROWS + 1, C], f32, kind="ExternalInput")
        for c in range(N)
    ]
    gidx_d = nc.dram_tensor("gidx", [PART, tot_tiles * 8], i16,
                            kind="ExternalInput")
    sidx_d = nc.dram_tensor("sidx", [PART, tot_w * 8], i16,
                            kind="ExternalInput")
    outs = [
        nc.dram_tensor(f"out{k}", [NSEG_CL + 1, C], f32,
                       kind="ExternalOutput")
        for k in range(NCLS)
    ]
    store_d = (
        nc.dram_tensor("partials", [tot_w * PART, C], f32,
                       kind="ExternalOutput")
        if "store" in parts
        else None
    )

    gb_cols = plan["max_tiles"] * C
    rb_cols = plan["max_nw"] * C

    with tile.TileContext(nc) as tc:
        with (
            tc.tile_pool(name="idxp", bufs=1) as ip,
            tc.tile_pool(name="gatp", bufs=LAG + 2) as gp,
            tc.tile_pool(name="resp", bufs=LAG + 2) as rp,
        ):
            gidx_sb = ip.tile([PART, tot_tiles * 8], i16)
            sidx_sb = ip.tile([PART, tot_w * 8], i16)
            # split the gidx load so the first gathers only wait on
            # their own slice (cuts single-shot pipeline-fill latency)
            head = min(plan["chunks"][1]["gt"] + plan["chunks"][1]["tiles"]
                       if len(plan["chunks"]) > 1 else tot_tiles,
                       tot_tiles) * 8
            nc.sync.dma_start(out=gidx_sb[:, :head], in_=gidx_d[:, :head])
            if head < tot_tiles * 8:
                nc.sync.dma_start(out=gidx_sb[:, head:],
                                  in_=gidx_d[:, head:])
            nc.sync.dma_start(out=sidx_sb[:], in_=sidx_d[:])

            def body():
                _emit(nc, tc, plan, xz, gidx_sb, sidx_sb, outs, gp, rp,
                      gb_cols, rb_cols, mybir, parts=parts, store_d=store_d,
                      queue_mode=queue_mode, max_chunks=max_chunks)

            if loop_reps:
                with tc.For_i(0, loop_reps, 1):
                    body()
            else:
                body()

    nc.compile()
    return nc


def _emit(nc, tc, plan, xz, gidx_sb, sidx_sb, outs, gp, rp,
          gb_cols, rb_cols, mybir, parts=("gather", "reduce", "scatter"),
          store_d=None, queue_mode="cls", max_chunks=None):
    from concourse import mybir as _mb
    f32 = _mb.dt.float32
    pend = []

    def qnum(ch):
        return 0 if queue_mode == "zero" else ch["cls"]

    def emit_scatter(ch, res):
        nw = ch["nw"]
        n_tok = nw * PART
        nc.gpsimd.dma_scatter_add(
            out_ap=outs[ch["cls"]][:, :],
            in_ap=res[:, : nw * C].rearrange("p (t c) -> p t c", t=nw, c=C),
            idxs_ap=sidx_sb[:, ch["gw"] * 8 : (ch["gw"] + nw) * 8],
            num_idxs=n_tok,
            num_idxs_reg=n_tok,
            elem_size=C,
            single_packet=True,
            queue_num=qnum(ch),
        )

    chunks = plan["chunks"]
    if max_chunks is not None:
        chunks = chunks[:max_chunks]
    for ch in chunks:
        T, nw = ch["tiles"], ch["nw"]
        buf = gp.tile([PART, gb_cols], f32, tag="gbuf")
        n_in = T * PART
        nc.gpsimd.dma_gather(
            out_ap=buf[:, : T * C].rearrange("p (t c) -> p t c", t=T, c=C),
            in_ap=xz[ch["cam"]][:, :],
            idxs_ap=gidx_sb[:, ch["gt"] * 8 : (ch["gt"] + T) * 8],
            num_idxs=n_in,
            num_idxs_reg=n_in,
            elem_size=C,
            single_packet=False,
            queue_num=qnum(ch),
        )
        if "reduce" not in parts:
            continue
        res = rp.tile([PART, rb_cols], f32, tag="rbuf")
        for L, _ws, nwr, toff, woff in ch["runs"]:
            view = buf[:, toff * C : (toff + nwr * L) * C].rearrange(
                "p (w j c) -> p w c j", w=nwr, j=L, c=C)
            nc.vector.tensor_reduce(
                out=res[:, woff * C : (woff + nwr) * C],
                in_=view,
                axis=mybir.AxisListType.X,
                op=mybir.AluOpType.add,
            )
        if "store" in parts:
            nw = ch["nw"]
            tot_w = plan["tot_w"]
            store_v = store_d[:, :].rearrange(
                "(p t) c -> p t c", p=PART, t=tot_w)
            nc.sync.dma_start(
                out=store_v[:, ch["gw"] : ch["gw"] + nw, :],
                in_=res[:, : nw * C].rearrange("p (t c) -> p t c", t=nw, c=C),
            )
        if "scatter" not in parts:
            continue
        pend.append((ch, res))
        if len(pend) > LAG:
            emit_scatter(*pend.pop(0))
    for ch, res in pend:
        emit_scatter(ch, res)


def _numpy_fallback(x, geom_feats):
    feats = np.asarray(x).reshape(-1, C)
    g = np.asarray(geom_feats).reshape(-1, 3).astype(np.int64)
    npr = feats.shape[0]
    batch_ix = np.repeat(np.arange(B, dtype=np.int64), npr // B)
    kept = (
        (g[:, 0] >= 0) & (g[:, 0] < NX)
        & (g[:, 1] >= 0) & (g[:, 1] < NY)
        & (g[:, 2] >= 0) & (g[:, 2] < NZ)
    )
    feats = np.where(kept[:, None], feats, 0.0)
    seg = batch_ix * NSEG + g[:, 0] * NY * NZ + g[:, 1] * NZ + g[:, 2]
    seg = np.where(kept, seg, batch_ix * NSEG)
    pooled = np.zeros((B * NSEG, C), np.float32)
    np.add.at(pooled, seg, feats)
    grid = pooled.reshape(B, NX, NY, NZ, C).transpose(0, 4, 3, 1, 2)
    return np.ascontiguousarray(grid.reshape(B, C * NZ, NX, NY))


def _make_in_maps(x, per_core):
    zero_row = np.zeros((1, C), np.float32)
    xz_by_batch = []
    for b in range(B):
        xb = x[b].reshape(N, CAM_ROWS, C)
        xz_by_batch.append(
            [np.concatenate([xb[c], zero_row], axis=0) for c in range(N)]
        )

    in_maps = []
    for pc in per_core:
        m = {f"xz{c}": xz_by_batch[pc["b"]][c] for c in range(N)}
        m["gidx"] = pc["gidx"]
        m["sidx"] = pc["sidx"]
        in_maps.append(m)
    return in_maps


def kernel(x, geom_feats):
    from concourse import bass_utils

    x = np.ascontiguousarray(np.asarray(x, dtype=np.float32))
    plan, cores, per_core = _host_layout(x, geom_feats)
    if plan is None:
        return _numpy_fallback(x, geom_feats)

    nc = _build_program(plan)
    in_maps = _make_in_maps(x, per_core)

    res = bass_utils.run_bass_kernel_spmd(
        nc, in_maps, core_ids=list(range(N_CORES))
    )
    global LAST_RESULTS
    LAST_RESULTS = res

    out = np.zeros((B, C * NZ, NX, NY), np.float32)
    half = np.empty((NSEG_H, C), np.float32)
    for pc, r in zip(per_core, res.results):
        for k in range(NCLS):
            half[k::NCLS] = r[f"out{k}"][:NSEG_CL]
        grid = half.reshape(NX // 2, NY, C)
        out[pc["b"], :, pc["h"] * (NX // 2) : (pc["h"] + 1) * (NX // 2), :] = (
            grid.transpose(2, 0, 1)
        )
    return out

